# revision 1
# baseline (speedup 1.0000x reference)
"""Trainium2 Bass kernel for the all-pairs cosine-similarity loss.

Reference computes:  loss = mean_{i<j}(1 - cos(f_i, f_j))
Closed form used here (mathematically identical for nonzero rows):
    u_i = f_i / ||f_i||           (normalized rows)
    g   = sum_i u_i               (D-vector)
    sum_{i<j} cos(f_i,f_j) = (||g||^2 - N) / 2
    loss = 1 - (||g||^2 - N) / (2 * num_pairs)

This turns an O(N^2 D) matmul problem into an O(N D) memory-bound pass:
each core streams its 512-row shard once (cast to bf16 during the DMA),
computes row norms (ACT square+accum), does a weighted row-sum on the
tensor engine (w = 1/||f_i|| stationary, bf16), AllGathers the 8
partial [1024] vectors, and finishes the scalar on-device.

bf16 note: the matmul operands are bf16 but every accumulation is fp32
(PSUM / accum_out).  The loss is 1 + O(1e-5) and the bf16 rounding of
unit-normalized rows perturbs it by ~1e-7 — far below the fp32
rounding noise of the reference's own 16M-element reduction.
"""

import numpy as np

N = 4096
D = 1024
N_CORES = 8
ROWS = N // N_CORES          # 512 rows per core
P = 128                      # SBUF partitions
T = ROWS // P                # 4 row-tiles of [128, D] per core
NUM_PAIRS = N * (N - 1) // 2

_LOSS_SCALE = -1.0 / (2.0 * NUM_PAIRS)
_LOSS_BIAS = 1.0 + N / (2.0 * NUM_PAIRS)

_built = None


def _build(collective: bool = True):
    import concourse.bacc as bacc
    import concourse.mybir as mybir
    import concourse.tile as tile

    f32 = mybir.dt.float32
    bf16 = mybir.dt.bfloat16
    nc = bacc.Bacc(
        "TRN2", target_bir_lowering=False, debug=False, num_devices=N_CORES
    )

    feats = nc.dram_tensor("feats", [ROWS, D], f32, kind="ExternalInput")
    loss_out = nc.dram_tensor("loss", [1, 1], f32, kind="ExternalOutput")
    # Internal DRAM bounce buffers for the collective (I/O tensors are not
    # legal collective operands; output must be in the Shared scratchpad).
    g_local = nc.dram_tensor("g_local", [1, D], bf16)
    g_all = nc.dram_tensor("g_all", [N_CORES, D], bf16, addr_space="Shared")

    with tile.TileContext(nc) as tc:
        with (
            tc.tile_pool(name="pool", bufs=1) as pool,
            tc.tile_pool(name="psum", bufs=1, space="PSUM") as psum,
        ):
            # Warm both ACT function-table sets (Square / Sqrt+Copy) while
            # the input DMAs stream — otherwise the 1.3us table load for
            # Sqrt lands on the critical path between squares and matmuls.
            dummy = pool.tile([1, 1], f32, tag="dummy")
            nc.gpsimd.memset(dummy[:], 1.0)
            nc.scalar.square(dummy[:], dummy[:])
            nc.scalar.sqrt(dummy[:], dummy[:])

            # Load + cast f32 -> bf16 during the DMA (SWDGE handles the
            # dtype conversion inline).
            fview = feats.ap().rearrange("(t p) d -> t p d", p=P)
            ftiles = []
            for t in range(T):
                ft = pool.tile([P, D], bf16, tag=f"f{t}", name=f"ft{t}")
                nc.gpsimd.dma_start(ft[:], fview[t])
                ftiles.append(ft)

            # Per-tile chains: square+rowsum (ACT, fp32 accum) -> sqrt (ACT)
            # -> reciprocal (DVE) -> bf16 cast (DVE) -> PE matmul pair.
            # Per-tile (not batched) so tile t's matmuls start as soon as
            # its own norm is ready instead of waiting on all 4 squares.
            # NB: vector.tensor_tensor_reduce crashes the NRT worker on
            # this runtime — keep to ACT/standard DVE instructions.
            sq = pool.tile([P, T], f32, tag="sq")
            nrm = pool.tile([P, T], f32, tag="nrm")
            w = pool.tile([P, T], f32, tag="w")
            wb = pool.tile([P, T], bf16, tag="wb")
            sc_a = pool.tile([P, D], bf16, tag="sc_a")
            gp = psum.tile([1, D], f32, tag="gp")
            for t in range(T):
                ts = slice(t, t + 1)
                nc.scalar.activation(
                    sc_a[:],
                    ftiles[t][:],
                    mybir.ActivationFunctionType.Square,
                    accum_out=sq[:, ts],
                )
                nc.scalar.sqrt(nrm[:, ts], sq[:, ts])
                nc.vector.reciprocal(w[:, ts], nrm[:, ts])
                nc.vector.tensor_copy(wb[:, ts], w[:, ts])
                for h in range(2):
                    nc.tensor.matmul(
                        gp[:, h * 512 : (h + 1) * 512],
                        wb[:, ts],
                        ftiles[t][:, h * 512 : (h + 1) * 512],
                        start=(t == 0),
                        stop=(t == T - 1),
                    )

            # PSUM -> SBUF (split across ACT+DVE, casting to bf16 so the
            # collective ships 2KB/rank) -> DRAM (dma_start can't source
            # PSUM). g ~ O(10) per component, so bf16 here costs ~1e-7 on
            # the final loss.
            gs = pool.tile([1, D], bf16, tag="gs")
            nc.scalar.copy(gs[:, 0:512], gp[:, 0:512])
            nc.vector.tensor_copy(gs[:, 512:D], gp[:, 512:D])
            nc.sync.dma_start(g_local.ap(), gs[:])

            if collective:
                nc.gpsimd.collective_compute(
                    "AllGather",
                    mybir.AluOpType.bypass,
                    replica_groups=[list(range(N_CORES))],
                    ins=[g_local.ap().opt()],
                    outs=[g_all.ap().opt()],
                )
            else:
                # timing-model variant (TimelineSim can't simulate
                # collectives): stand-in DMA with the same data deps
                nc.sync.dma_start(g_all.ap()[0:1], g_local.ap())

            # Bring the 8 partials in as [8, D] bf16 (cast on load),
            # reduce ranks on PE with a ones vector, square-reduce on ACT.
            ga = pool.tile([N_CORES, D], bf16, tag="ga")
            nc.gpsimd.dma_start(ga[:], g_all.ap())
            ones8 = pool.tile([N_CORES, 1], bf16, tag="ones8")
            nc.gpsimd.memset(ones8[:], 1.0)

            gt = psum.tile([1, D], f32, tag="gt")
            for h in range(2):
                nc.tensor.matmul(
                    gt[:, h * 512 : (h + 1) * 512],
                    ones8[:],
                    ga[:, h * 512 : (h + 1) * 512],
                    start=True,
                    stop=True,
                )

            sc_g = pool.tile([1, D], f32, tag="sc_g")
            gg = pool.tile([1, 1], f32, tag="gg")
            nc.scalar.activation(
                sc_g[:],
                gt[:],
                mybir.ActivationFunctionType.Square,
                accum_out=gg[:],
            )

            # loss = 1 - (gg - N) / (2*num_pairs)  ==  gg*scale + bias
            loss_sb = pool.tile([1, 1], f32, tag="loss_sb")
            nc.scalar.activation(
                loss_sb[:],
                gg[:],
                mybir.ActivationFunctionType.Copy,
                bias=_LOSS_BIAS,
                scale=_LOSS_SCALE,
            )
            nc.sync.dma_start(loss_out.ap(), loss_sb[:])

    nc.compile()
    return nc


def _get_nc():
    global _built
    if _built is None:
        _built = _build()
    return _built


def kernel(feats: np.ndarray) -> np.ndarray:
    from concourse import bass_utils

    nc = _get_nc()
    feats = np.ascontiguousarray(np.asarray(feats, dtype=np.float32))
    assert feats.shape == (N, D), feats.shape

    in_maps = [
        {"feats": feats[c * ROWS : (c + 1) * ROWS]} for c in range(N_CORES)
    ]
    res = bass_utils.run_bass_kernel_spmd(
        nc, in_maps, core_ids=list(range(N_CORES))
    )
    out = res.results[0]["loss"]
    return np.float32(out.reshape(())[()])



# revision 4
# speedup vs baseline: 1.0715x; 1.0715x over previous
"""Trainium2 Bass kernel for the all-pairs cosine-similarity loss.

Reference computes:  loss = mean_{i<j}(1 - cos(f_i, f_j))
Closed form used here (mathematically identical for nonzero rows):
    u_i = f_i / ||f_i||           (normalized rows)
    g   = sum_i u_i               (D-vector)
    sum_{i<j} cos(f_i,f_j) = (||g||^2 - N) / 2
    loss = 1 - (||g||^2 - N) / (2 * num_pairs)

O(N D) memory-bound pass, 512 rows per core. Key layout choice vs the
earlier revision: the per-core partial g is produced PARTITION-SPREAD
([128, 8] with d = 128*j + p) by making the FEATURE tile the stationary
matmul operand (out = ftile^T @ w).  That kills every single-partition
op in the tail: the PSUM->SBUF copy is [128,8] (one cheap DVE op), the
AllGather payload is the same 2KB, and the post-gather reduce is 8
feature-stationary [8,128] matmuls + a [128,8] square instead of a
1038ns single-partition ACT pass.

Squares run on DVE via scalar_tensor_tensor (one fused mult+row-accum
pass per tile) so the ACT engine no longer serializes 4x1038ns; input
streams in 2 big SWDGE cast-DMAs (f32->bf16) instead of 4 (desc-gen on
the gpsimd engine was the issue-rate limiter).

bf16 note: matmul operands are bf16, every accumulation is fp32.  The
loss error budget is enormous (loss ~ 1 + 2e-5; abs tolerance 2e-4 on a
~1.0 value), bf16 rounding lands ~1e-7.
"""

import numpy as np

N = 4096
D = 1024
N_CORES = 8
ROWS = N // N_CORES          # 512 rows per core
P = 128                      # SBUF partitions
T = ROWS // P                # 4 row-tiles of [128, D] per core
NCH = 8                      # 128-col chunks per tile (matmul stationary)
NUM_PAIRS = N * (N - 1) // 2

_LOSS_SCALE = -1.0 / (2.0 * NUM_PAIRS)
_LOSS_BIAS = 1.0 + N / (2.0 * NUM_PAIRS)

_built = None


def _build(collective: bool = True):
    import concourse.bacc as bacc
    import concourse.mybir as mybir
    import concourse.tile as tile

    f32 = mybir.dt.float32
    bf16 = mybir.dt.bfloat16
    nc = bacc.Bacc(
        "TRN2", target_bir_lowering=False, debug=False, num_devices=N_CORES
    )

    feats = nc.dram_tensor("feats", [ROWS, D], f32, kind="ExternalInput")
    loss_out = nc.dram_tensor("loss", [1, 1], f32, kind="ExternalOutput")
    # Internal DRAM bounce buffers for the collective (I/O tensors are not
    # legal collective operands; output must be in the Shared scratchpad).
    g_local = nc.dram_tensor("g_local", [P, NCH], bf16)
    g_all = nc.dram_tensor("g_all", [N_CORES, P, NCH], bf16, addr_space="Shared")

    with tile.TileContext(nc) as tc:
        with (
            tc.tile_pool(name="pool", bufs=1) as pool,
            tc.tile_pool(name="psum", bufs=1, space="PSUM") as psum,
        ):
            # Warm both ACT function-table sets (Square / Sqrt+Copy) while
            # the input DMAs stream — a 1.3us table load otherwise lands on
            # the critical path before the first sqrt.
            dummy = pool.tile([1, 1], f32, tag="dummy")
            nc.gpsimd.memset(dummy[:], 1.0)
            nc.scalar.square(dummy[:], dummy[:])
            nc.scalar.sqrt(dummy[:], dummy[:])

            # Load + cast f32 -> bf16 during the DMA (SWDGE casts inline).
            # 2 chunk DMAs of 2 tiles each: desc-gen on the gpsimd engine is
            # ~1us per dma_start, so 4 separate tile loads were issue-bound.
            fview = feats.ap().rearrange("(c t p) d -> c p t d", c=2, t=2, p=P)
            fch = []
            for c in range(2):
                ft = pool.tile([P, 2 * D], bf16, tag=f"f{c}", name=f"fch{c}")
                nc.gpsimd.dma_start(ft[:], fview[c])
                fch.append(ft)

            ones128 = pool.tile([P, 1], bf16, tag="ones128")
            nc.gpsimd.memset(ones128[:], 1.0)
            ones8 = pool.tile([N_CORES, 1], bf16, tag="ones8")
            nc.gpsimd.memset(ones8[:], 1.0)

            # Per-tile: fused square+rowsum on DVE (scalar_tensor_tensor:
            # out = (x * 1) * x, accum_out = row sum) -> sqrt (ACT) ->
            # reciprocal (DVE) -> bf16 cast (DVE) -> 8 feature-stationary
            # matmuls (ftile chunk [128,128] stationary, w [128,1] moving)
            # accumulating g into PSUM [128, 8], d = 128*j + p.
            sq = pool.tile([P, T], f32, tag="sq")
            nrm = pool.tile([P, T], f32, tag="nrm")
            w = pool.tile([P, T], f32, tag="w")
            wb = pool.tile([P, T], bf16, tag="wb")
            sc = pool.tile([P, D], bf16, tag="sc")
            gp = psum.tile([P, NCH], f32, tag="gp")
            for t in range(T):
                c, ti = divmod(t, 2)
                ftile = fch[c][:, ti * D : (ti + 1) * D]
                ts = slice(t, t + 1)
                nc.vector.scalar_tensor_tensor(
                    sc[:],
                    ftile,
                    1.0,
                    ftile,
                    mybir.AluOpType.mult,
                    mybir.AluOpType.mult,
                    accum_out=sq[:, ts],
                )
                nc.scalar.sqrt(nrm[:, ts], sq[:, ts])
                nc.vector.reciprocal(w[:, ts], nrm[:, ts])
                nc.vector.tensor_copy(wb[:, ts], w[:, ts])
                for j in range(NCH):
                    nc.tensor.matmul(
                        gp[:, j : j + 1],
                        ftile[:, j * P : (j + 1) * P],
                        wb[:, ts],
                        start=(t == 0),
                        stop=(t == T - 1),
                    )

            # PSUM -> SBUF (one [128,8] copy, bf16 so the collective ships
            # 2KB/rank) -> DRAM.
            gs = pool.tile([P, NCH], bf16, tag="gs")
            nc.vector.tensor_copy(gs[:], gp[:])
            nc.sync.dma_start(g_local.ap(), gs[:])

            if collective:
                nc.gpsimd.collective_compute(
                    "AllGather",
                    mybir.AluOpType.bypass,
                    replica_groups=[list(range(N_CORES))],
                    ins=[g_local.ap().opt()],
                    outs=[g_all.ap().opt()],
                )
            else:
                # timing-model variant (TimelineSim can't simulate
                # collectives): stand-in DMA with the same data deps
                nc.sync.dma_start(g_all.ap()[0:1], g_local.ap())

            # Load the 8 partials as [8, 1024] (8 contiguous 2KB rows — 8
            # DMA descriptors).  Rank r, free index p*8+j holds g_r[128j+p].
            # Rank-sum via 8 feature-stationary matmuls: stationary
            # [8, 128] (stride-8 column view), moving ones8 [8,1] ->
            # gsum [128, 8] f32 in PSUM, same d = 128*j + p layout.
            ga = pool.tile([N_CORES, P * NCH], bf16, tag="ga")
            nc.sync.dma_start(ga[:], g_all.ap().rearrange("r p j -> r (p j)"))
            gav = ga[:].rearrange("r (p j) -> j r p", j=NCH)
            gt = psum.tile([P, NCH], f32, tag="gt")
            for j in range(NCH):
                nc.tensor.matmul(
                    gt[:, j : j + 1],
                    gav[j],
                    ones8[:],
                    start=True,
                    stop=True,
                )

            # ||g||^2: fused square+rowsum on ACT ([128,8] is ~8 cols, so
            # this is cheap here), then cross-partition sum on PE.
            gsq = pool.tile([P, NCH], bf16, tag="gsq")
            ssq = pool.tile([P, 1], f32, tag="ssq")
            nc.scalar.activation(
                gsq[:],
                gt[:],
                mybir.ActivationFunctionType.Square,
                accum_out=ssq[:],
            )
            ssqb = pool.tile([P, 1], bf16, tag="ssqb")
            nc.vector.tensor_copy(ssqb[:], ssq[:])
            lp = psum.tile([1, 1], f32, tag="lp")
            nc.tensor.matmul(lp[:], ssqb[:], ones128[:], start=True, stop=True)

            # loss = 1 - (gg - N) / (2*num_pairs)  ==  gg*scale + bias
            loss_sb = pool.tile([1, 1], f32, tag="loss_sb")
            nc.scalar.activation(
                loss_sb[:],
                lp[:],
                mybir.ActivationFunctionType.Copy,
                bias=_LOSS_BIAS,
                scale=_LOSS_SCALE,
            )
            nc.sync.dma_start(loss_out.ap(), loss_sb[:])

    nc.compile()
    return nc


def _get_nc():
    global _built
    if _built is None:
        _built = _build()
    return _built


def kernel(feats: np.ndarray) -> np.ndarray:
    from concourse import bass_utils

    nc = _get_nc()
    feats = np.ascontiguousarray(np.asarray(feats, dtype=np.float32))
    assert feats.shape == (N, D), feats.shape

    in_maps = [
        {"feats": feats[c * ROWS : (c + 1) * ROWS]} for c in range(N_CORES)
    ]
    res = bass_utils.run_bass_kernel_spmd(
        nc, in_maps, core_ids=list(range(N_CORES))
    )
    out = res.results[0]["loss"]
    return np.float32(out.reshape(())[()])


# revision 6
# speedup vs baseline: 1.0873x; 1.0147x over previous
"""Trainium2 Bass kernel for the all-pairs cosine-similarity loss.

Reference computes:  loss = mean_{i<j}(1 - cos(f_i, f_j))
Closed form used here (mathematically identical for nonzero rows):
    u_i = f_i / ||f_i||           (normalized rows)
    g   = sum_i u_i               (D-vector)
    sum_{i<j} cos(f_i,f_j) = (||g||^2 - N) / 2
    loss = 1 - (||g||^2 - N) / (2 * num_pairs)

O(N D) memory-bound pass, 512 rows per core. Key layout choice vs the
earlier revision: the per-core partial g is produced PARTITION-SPREAD
([128, 8] with d = 128*j + p) by making the FEATURE tile the stationary
matmul operand (out = ftile^T @ w).  That kills every single-partition
op in the tail: the PSUM->SBUF copy is [128,8] (one cheap DVE op), the
AllGather payload is the same 2KB, and the post-gather reduce is 8
feature-stationary [8,128] matmuls + a [128,8] square instead of a
1038ns single-partition ACT pass.

Squares run on DVE via scalar_tensor_tensor (one fused mult+row-accum
pass per tile) so the ACT engine no longer serializes 4x1038ns; input
streams in 2 big SWDGE cast-DMAs (f32->bf16) instead of 4 (desc-gen on
the gpsimd engine was the issue-rate limiter).

bf16 note: matmul operands are bf16, every accumulation is fp32.  The
loss error budget is enormous (loss ~ 1 + 2e-5; abs tolerance 2e-4 on a
~1.0 value), bf16 rounding lands ~1e-7.
"""

import numpy as np

N = 4096
D = 1024
N_CORES = 8
ROWS = N // N_CORES          # 512 rows per core
P = 128                      # SBUF partitions
T = ROWS // P                # 4 row-tiles of [128, D] per core
NCH = 8                      # 128-col chunks per tile (matmul stationary)
NUM_PAIRS = N * (N - 1) // 2

_LOSS_SCALE = -1.0 / (2.0 * NUM_PAIRS)
_LOSS_BIAS = 1.0 + N / (2.0 * NUM_PAIRS)

_built = None


def _build(collective: bool = True):
    import concourse.bacc as bacc
    import concourse.mybir as mybir
    import concourse.tile as tile

    f32 = mybir.dt.float32
    bf16 = mybir.dt.bfloat16
    nc = bacc.Bacc(
        "TRN2", target_bir_lowering=False, debug=False, num_devices=N_CORES
    )

    feats = nc.dram_tensor("feats", [ROWS, D], f32, kind="ExternalInput")
    loss_out = nc.dram_tensor("loss", [1, 1], f32, kind="ExternalOutput")
    # Internal DRAM bounce buffers for the collective (I/O tensors are not
    # legal collective operands; output must be in the Shared scratchpad).
    g_local = nc.dram_tensor("g_local", [P, NCH], bf16)
    g_all = nc.dram_tensor("g_all", [N_CORES, P, NCH], bf16, addr_space="Shared")

    with tile.TileContext(nc) as tc:
        with (
            tc.tile_pool(name="pool", bufs=1) as pool,
            tc.tile_pool(name="psum", bufs=1, space="PSUM") as psum,
        ):
            # Load + cast f32 -> bf16 during the DMA (SWDGE casts inline).
            # 2 chunk DMAs of 2 tiles each: desc-gen on the gpsimd engine is
            # ~1us per dma_start, so 4 separate tile loads were issue-bound.
            # Issued before anything else so desc-gen starts immediately.
            fview = feats.ap().rearrange("(c t p) d -> c p t d", c=2, t=2, p=P)
            fch = []
            for c in range(2):
                ft = pool.tile([P, 2 * D], bf16, tag=f"f{c}", name=f"fch{c}")
                nc.gpsimd.dma_start(ft[:], fview[c])
                fch.append(ft)

            # Warm both ACT function-table sets (Square / Sqrt+Copy) while
            # the input DMAs stream — a 1.3us table load otherwise lands on
            # the critical path before the first square.
            dummy = pool.tile([1, 1], f32, tag="dummy")
            nc.gpsimd.memset(dummy[:], 1.0)
            nc.scalar.square(dummy[:], dummy[:])
            nc.scalar.sqrt(dummy[:], dummy[:])

            ones128 = pool.tile([P, 1], bf16, tag="ones128")
            nc.gpsimd.memset(ones128[:], 1.0)
            ones8 = pool.tile([N_CORES, 1], bf16, tag="ones8")
            nc.gpsimd.memset(ones8[:], 1.0)

            # Per-tile: fused square+rowsum on DVE (scalar_tensor_tensor:
            # out = (x * 1) * x, accum_out = row sum) -> sqrt (ACT) ->
            # reciprocal (DVE) -> bf16 cast (DVE) -> 8 feature-stationary
            # matmuls (ftile chunk [128,128] stationary, w [128,1] moving)
            # accumulating g into PSUM [128, 8], d = 128*j + p.
            # Squares split ~50/50 ACT/DVE per tile (both engines charge
            # ~1.1ns/col for a [128,c] pass, so one engine alone serializes
            # 4x1.1us).  ACT squares cols [0:H) with row-accum -> sqA; DVE
            # does cols [H:D) in one fused mult+accum pass -> sqB; the sqrt
            # then fuses the partial-sum add via its per-partition bias AP:
            # nrm = sqrt(sqA*1 + sqB).
            H = D // 2
            sqa = pool.tile([P, T], f32, tag="sqa")
            sqb = pool.tile([P, T], f32, tag="sqb")
            nrm = pool.tile([P, T], f32, tag="nrm")
            w = pool.tile([P, T], f32, tag="w")
            wb = pool.tile([P, T], bf16, tag="wb")
            sc = pool.tile([P, D], bf16, tag="sc")
            gp = psum.tile([P, NCH], f32, tag="gp")
            for t in range(T):
                c, ti = divmod(t, 2)
                ftile = fch[c][:, ti * D : (ti + 1) * D]
                ts = slice(t, t + 1)
                nc.scalar.activation(
                    sc[:, 0:H],
                    ftile[:, 0:H],
                    mybir.ActivationFunctionType.Square,
                    accum_out=sqa[:, ts],
                )
                nc.vector.scalar_tensor_tensor(
                    sc[:, H:D],
                    ftile[:, H:D],
                    1.0,
                    ftile[:, H:D],
                    mybir.AluOpType.mult,
                    mybir.AluOpType.mult,
                    accum_out=sqb[:, ts],
                )
                nc.scalar.activation(
                    nrm[:, ts],
                    sqa[:, ts],
                    mybir.ActivationFunctionType.Sqrt,
                    bias=sqb[:, ts],
                )
                nc.vector.reciprocal(w[:, ts], nrm[:, ts])
                nc.vector.tensor_copy(wb[:, ts], w[:, ts])
                for j in range(NCH):
                    nc.tensor.matmul(
                        gp[:, j : j + 1],
                        ftile[:, j * P : (j + 1) * P],
                        wb[:, ts],
                        start=(t == 0),
                        stop=(t == T - 1),
                    )

            # PSUM -> SBUF (one [128,8] copy, bf16 so the collective ships
            # 2KB/rank) -> DRAM.
            gs = pool.tile([P, NCH], bf16, tag="gs")
            nc.vector.tensor_copy(gs[:], gp[:])
            nc.sync.dma_start(g_local.ap(), gs[:])

            if collective:
                nc.gpsimd.collective_compute(
                    "AllGather",
                    mybir.AluOpType.bypass,
                    replica_groups=[list(range(N_CORES))],
                    ins=[g_local.ap().opt()],
                    outs=[g_all.ap().opt()],
                )
            else:
                # timing-model variant (TimelineSim can't simulate
                # collectives): stand-in DMA with the same data deps
                nc.sync.dma_start(g_all.ap()[0:1], g_local.ap())

            # Load the 8 partials as [8, 1024] (8 contiguous 2KB rows — 8
            # DMA descriptors).  Rank r, free index p*8+j holds g_r[128j+p].
            # Rank-sum via 8 feature-stationary matmuls: stationary
            # [8, 128] (stride-8 column view), moving ones8 [8,1] ->
            # gsum [128, 8] f32 in PSUM, same d = 128*j + p layout.
            ga = pool.tile([N_CORES, P * NCH], bf16, tag="ga")
            nc.sync.dma_start(ga[:], g_all.ap().rearrange("r p j -> r (p j)"))
            gav = ga[:].rearrange("r (p j) -> j r p", j=NCH)
            gt = psum.tile([P, NCH], f32, tag="gt")
            for j in range(NCH):
                nc.tensor.matmul(
                    gt[:, j : j + 1],
                    gav[j],
                    ones8[:],
                    start=True,
                    stop=True,
                )

            # ||g||^2: fused square+rowsum on ACT ([128,8] is ~8 cols, so
            # this is cheap here), then cross-partition sum on PE.
            gsq = pool.tile([P, NCH], bf16, tag="gsq")
            ssq = pool.tile([P, 1], f32, tag="ssq")
            nc.scalar.activation(
                gsq[:],
                gt[:],
                mybir.ActivationFunctionType.Square,
                accum_out=ssq[:],
            )
            ssqb = pool.tile([P, 1], bf16, tag="ssqb")
            nc.vector.tensor_copy(ssqb[:], ssq[:])
            lp = psum.tile([1, 1], f32, tag="lp")
            nc.tensor.matmul(lp[:], ssqb[:], ones128[:], start=True, stop=True)

            # loss = 1 - (gg - N) / (2*num_pairs)  ==  gg*scale + bias
            loss_sb = pool.tile([1, 1], f32, tag="loss_sb")
            nc.scalar.activation(
                loss_sb[:],
                lp[:],
                mybir.ActivationFunctionType.Copy,
                bias=_LOSS_BIAS,
                scale=_LOSS_SCALE,
            )
            nc.sync.dma_start(loss_out.ap(), loss_sb[:])

    nc.compile()
    return nc


def _get_nc():
    global _built
    if _built is None:
        _built = _build()
    return _built


def kernel(feats: np.ndarray) -> np.ndarray:
    from concourse import bass_utils

    nc = _get_nc()
    feats = np.ascontiguousarray(np.asarray(feats, dtype=np.float32))
    assert feats.shape == (N, D), feats.shape

    in_maps = [
        {"feats": feats[c * ROWS : (c + 1) * ROWS]} for c in range(N_CORES)
    ]
    res = bass_utils.run_bass_kernel_spmd(
        nc, in_maps, core_ids=list(range(N_CORES))
    )
    out = res.results[0]["loss"]
    return np.float32(out.reshape(())[()])
